# revision 70
# baseline (speedup 1.0000x reference)
"""Complex-valued multi-head attention (B=4, S=1024, D=128, H=8) on 8 TRN2 cores.

Sharding: tensor-parallel over heads -- one head per NeuronCore. Each core
computes its head's complex Q/K/V projections, complex-magnitude-softmax
attention, and the partial W_o projection for its head slice; the host sums
the 8 partial outputs (the W_o contraction over heads).

Per-core dataflow (fp32 storage, float32r matmuls -- 1 cycle/row on the PE
at moving free-dim >= 256).  The Tile scheduler executes each engine's
instructions out of program order (priority heap over READY instructions),
so the design controls *dependencies and priorities*, not issue order:
  - Host packs x^T tensors [D, B*S] and per-head weight slices (transposed,
    pre-negated / concatenated) into two DRAM tensors (xall, wpack).
    x^T half-batches are prefetched one batch ahead (3 parallel DMA queues).
  - Q^T, K^T computed as [d, {r,i}, s] via lhsT=W^T, rhs=x^T; -K_i^T by a
    Pool negate.  V computed natural [s, d] as [V_r | V_i] rows.
  - Scores computed transposed S^T[k, q] so E = exp(|s|/sqrt(D)) lands in
    the layout attn@V needs.  Per-tile egress (GPSIMD cannot touch PSUM, so
    all PSUM reads must go through ACT/DVE): ACT squares sr straight into
    the strip while DVE copies si out and Pool squares+adds; DVE_SQ tiles
    route sr through DVE+Pool instead, balancing ACT ~= DVE ~= 105us, both
    just under PE ~112us.
  - sqrt/exp of batch b are deferred into batch b+1's window as half-strip
    chunks issued at artificially LATE scheduler priority (+300), so ready
    score-squares preempt them and the chunks backfill ACT idle time
    instead of blocking the score pipeline.
  - attn@V computed TRANSPOSED: O^T[d, q] accumulates via lhsT=V-chunk
    (stationary), rhs=E-strip; the normalizer Z[q] comes from tiny [q,2]
    matmuls with the E-chunk stationary against a ones pair (~4 PE cycles
    each), kc-major.  No PE transposes, no osb/ocat staging copies.
  - O^T borrows ma/mb PSUM slots so the score pipeline keeps 3-deep
    buffering (8 banks: 3*ma + 3*mb + z + y).  attn for batch b-2 is issued
    BEFORE scores(b) -- issuing it after creates an ACT<->PE deadlock cycle
    through the strip-buffer rotation and the shared PSUM slots.
  - W_o (lhsT=O^T slices) is deferred one attn unit so its PSUM->SBUF
    egress never gates the next unit's O^T accumulation; 1/Z is folded
    into the final egress via per-partition tensor_scalar.

Known-dead ends (measured): fp8 scores fail accuracy (8e-2); fp8 hi/lo
scores pass accuracy (2.6e-3) but the lo-extraction adds +21us of DVE PSUM
egress, exceeding the PE savings; fp8 hi/lo projections fail accuracy
(1.9e-2); bf16 m2/logit strips fail margin (1.6e-2); fused [P,1024]
double-bank egress starves the PE (PSUM depth drops to 1).
"""

import ml_dtypes
import numpy as np

import concourse.bacc as bacc
import concourse.mybir as mybir
import concourse.tile as tile
from concourse.bass_utils import run_bass_kernel_spmd

B, S, D, H = 4, 1024, 128, 8
BS = B * S
P = 128
F32 = mybir.dt.float32
F32R = mybir.dt.float32r
BF16 = mybir.dt.bfloat16

X_NAMES = ("xqr", "xqi", "xkr", "xki", "xvr", "xvi")
W1_NAMES = ("wqr", "wqi", "nwqi", "wkr", "wki", "nwki")
W2_NAMES = ("vc1", "vc2", "oc1", "oc2")


W1_OFF = {nm: i * P for i, nm in enumerate(W1_NAMES)}
W2_OFF = {nm: 6 * P + i * 2 * P for i, nm in enumerate(W2_NAMES)}
WPACK_COLS = 6 * P + 4 * 2 * P      # 1792

# score tiles whose sr egress goes DVE+Pool instead of ACT (load balance)
DVE_SQ = {(0, 2), (0, 4), (1, 4)}


def build_nc():
    nc = bacc.Bacc()
    xall = nc.dram_tensor("xall", [P, 8 * 6 * 512], BF16, kind="ExternalInput")
    wpack = nc.dram_tensor("wpack", [P, WPACK_COLS], BF16, kind="ExternalInput")
    y = nc.dram_tensor("y", [BS, 2 * P], F32, kind="ExternalOutput")
    xv = xall.rearrange("p (t n c) -> p t n c", t=8, n=6)

    AF = mybir.ActivationFunctionType
    MUL = mybir.AluOpType.mult

    with tile.TileContext(nc) as tc:
        with (
            tc.tile_pool(name="consts", bufs=1) as consts,
            tc.tile_pool(name="xp", bufs=3) as xp,
            tc.tile_pool(name="ep", bufs=2) as ep,
            tc.tile_pool(name="qk", bufs=2) as qk,
            tc.tile_pool(name="vp", bufs=3) as vp,
            tc.tile_pool(name="sp", bufs=2) as sp,
            tc.tile_pool(name="sc", bufs=5) as sc,
            tc.tile_pool(name="ot", bufs=2) as otp,
            tc.tile_pool(name="yp", bufs=2) as yp,
            tc.tile_pool(name="rp", bufs=4) as rp,
            tc.tile_pool(name="ps", bufs=3, space="PSUM") as ps,
            tc.tile_pool(name="pz", bufs=1, space="PSUM") as pzp,
            tc.tile_pool(name="py", bufs=1, space="PSUM") as pyp,
        ):
            wp = consts.tile([P, WPACK_COLS], BF16, name="wp")
            xt_tiles = {}

            def xload(b, t2):
                """Prefetch one half-batch of x^T (3 DMAs -> parallel queues)."""
                xt = xp.tile([P, 6, 512], BF16, name="xt", tag="xt")
                for dd in range(3):
                    nc.sync.dma_start(
                        xt[:, 2 * dd : 2 * dd + 2, :],
                        xv[:, b * 2 + t2, 2 * dd : 2 * dd + 2, :],
                    )
                xt_tiles[(b, t2)] = xt

            xt0 = xp.tile([P, 6, 512], BF16, name="xt", tag="xt")
            nc.sync.dma_start(xt0[:, 0:2, :], xv[:, 0, 0:2, :])
            nc.sync.dma_start(wp[:, 0:768], wpack[:, 0:768])
            # V/O weights go out on the idle ACT queue so they don't push
            # the x prefetches back (each issue occupies its engine ~1.6us)
            nc.scalar.dma_start(wp[:, 768:WPACK_COLS], wpack[:, 768:WPACK_COLS])
            nc.sync.dma_start(xt0[:, 2:4, :], xv[:, 0, 2:4, :])
            nc.sync.dma_start(xt0[:, 4:6, :], xv[:, 0, 4:6, :])
            xt_tiles[(0, 0)] = xt0
            xload(0, 1)
            wt = {nm: wp[:, off : off + P] for nm, off in W1_OFF.items()}
            wt.update({nm: wp[:, off : off + 2 * P] for nm, off in W2_OFF.items()})
            onesb = consts.tile([P, 2], BF16, name="onesb")
            nc.vector.memset(onesb, 1.0)

            qc_all, kc_all, nki_all, v_all = {}, {}, {}, {}
            act_pend = []   # deferred ACT chunks (sqrt/exp 4-slice pieces)
            wo_pend = []    # deferred Wo+ybuf stages

            def pop_act(n=1):
                for _ in range(n):
                    if act_pend:
                        act_pend.pop(0)()

            def proj(b):
                # qcat/kcat: [d, {r,i}, s] per-batch projection outputs
                qcat = qk.tile([P, 2, S], BF16, name="qcat", tag="qcat")
                kcat = qk.tile([P, 2, S], BF16, name="kcat", tag="kcat")
                nkiT = qk.tile([P, S], BF16, name="nkiT", tag="nkiT")
                vcat = vp.tile([P, 8, 256], BF16, name="vcat", tag="vcat")
                for t2 in range(2):
                    cols = slice(t2 * 512, (t2 + 1) * 512)
                    xt = xt_tiles.pop((b, t2))
                    xqr, xqi = xt[:, 0, :], xt[:, 1, :]
                    xkr, xki = xt[:, 2, :], xt[:, 3, :]
                    xvr, xvi = xt[:, 4, :], xt[:, 5, :]

                    pqr = ps.tile([P, 512], F32, name="pqr", tag="ma")
                    nc.tensor.matmul(pqr, wt["wqr"], xqr, start=True, stop=False)
                    nc.tensor.matmul(pqr, wt["nwqi"], xqi, start=False, stop=True)
                    nc.vector.tensor_copy(qcat[:, 0, cols], pqr)
                    pqi = ps.tile([P, 512], F32, name="pqi", tag="mb")
                    nc.tensor.matmul(pqi, wt["wqi"], xqr, start=True, stop=False)
                    nc.tensor.matmul(pqi, wt["wqr"], xqi, start=False, stop=True)
                    nc.vector.tensor_copy(qcat[:, 1, cols], pqi)

                    pkr = ps.tile([P, 512], F32, name="pkr", tag="ma")
                    nc.tensor.matmul(pkr, wt["wkr"], xkr, start=True, stop=False)
                    nc.tensor.matmul(pkr, wt["nwki"], xki, start=False, stop=True)
                    nc.vector.tensor_copy(kcat[:, 0, cols], pkr)
                    pki = ps.tile([P, 512], F32, name="pki", tag="mb")
                    nc.tensor.matmul(pki, wt["wki"], xkr, start=True, stop=False)
                    nc.tensor.matmul(pki, wt["wkr"], xki, start=False, stop=True)
                    nc.vector.tensor_copy(kcat[:, 1, cols], pki)

                    nc.gpsimd.tensor_scalar_mul(nkiT[:, cols], kcat[:, 1, cols], -1.0)

                    for c2 in range(2):
                        gc = t2 * 4 + c2 * 2
                        pv = ps.tile([P, 512], F32, name="pv", tag="ma" if c2 == 0 else "mb")
                        for j in range(2):
                            cc = slice((c2 * 2 + j) * 128, (c2 * 2 + j + 1) * 128)
                            h = slice(j * 256, (j + 1) * 256)
                            nc.tensor.matmul(pv[:, h], xvr[:, cc], wt["vc1"], start=True, stop=False)
                            nc.tensor.matmul(pv[:, h], xvi[:, cc], wt["vc2"], start=False, stop=True)
                        nc.vector.tensor_copy(
                            vcat[:, gc : gc + 2, :],
                            pv.rearrange("p (a c) -> p a c", a=2),
                        )
                    if t2 == 1:
                        pop_act()
                qc_all[b] = qcat
                kc_all[b] = kcat
                nki_all[b] = nkiT
                v_all[b] = vcat

            def scores(b, qt, strip, pops=(1, 3, 5)):
                qcat, kcat, nkiT = qc_all[b], kc_all[b], nki_all[b]
                qcols = slice(qt * 512, (qt + 1) * 512)
                for kc in range(8):
                    sl8 = qt * 8 + kc
                    kcols = slice(kc * 128, (kc + 1) * 128)
                    psr = ps.tile([P, 512], F32, name="psr", tag="ma")
                    nc.tensor.matmul(psr, kcat[:, 0, kcols], qcat[:, 0, qcols], start=True, stop=False)
                    nc.tensor.matmul(psr, nkiT[:, kcols], qcat[:, 1, qcols], start=False, stop=True)
                    psi = ps.tile([P, 512], F32, name="psi", tag="mb")
                    nc.tensor.matmul(psi, kcat[:, 1, kcols], qcat[:, 0, qcols], start=True, stop=False)
                    nc.tensor.matmul(psi, kcat[:, 0, kcols], qcat[:, 1, qcols], start=False, stop=True)

                    cpi = sc.tile([P, 512], F32, name="cpi", tag="cpi")
                    nc.vector.tensor_copy(cpi, psi)
                    t2s = sc.tile([P, 512], F32, name="t2s", tag="t2s")
                    nc.gpsimd.tensor_mul(t2s, cpi, cpi)
                    if (qt, kc) in DVE_SQ:
                        cpr = sc.tile([P, 512], F32, name="cpr", tag="cpr")
                        nc.vector.tensor_copy(cpr, psr)
                        nc.gpsimd.tensor_mul(strip[:, sl8, :], cpr, cpr)
                    else:
                        nc.scalar.square(strip[:, sl8, :], psr)
                    nc.gpsimd.tensor_add(strip[:, sl8, :], strip[:, sl8, :], t2s)
                    if kc in pops:
                        pop_act()

            def attn_front(b, qt, estrip):
                vcat = v_all[b]
                # Z[q] per qc-chunk: tiny [q,2] matmuls, E-chunk stationary,
                # kc-major so they chase the exp chunks.  kc=0 relies on the
                # first start=True pending-zeroing the whole bank.
                pzt = pzp.tile([P, 8], F32, name="pzt", tag="z")
                for kc in range(8):
                    for qc in range(4):
                        nc.tensor.matmul(
                            pzt[:, 2 * qc : 2 * qc + 2],
                            estrip[:, qt * 8 + kc, qc * 128 : (qc + 1) * 128],
                            onesb,
                            start=(qc == 0 and kc == 0),
                            stop=(kc == 7),
                            skip_group_check=True,
                        )
                rec4 = rp.tile([P, 8], F32, name="rec4", tag="rec")
                nc.vector.reciprocal(rec4, pzt)

                # O^T[d, q] accumulation, V-chunk stationary; the two
                # complex halves borrow one ma and one mb slot so the score
                # pipeline can keep 3-deep PSUM buffering
                por = ps.tile([P, 512], F32, name="por", tag="ma")
                for kc in range(8):
                    nc.tensor.matmul(
                        por, vcat[:, kc, 0:128], estrip[:, qt * 8 + kc, :],
                        start=(kc == 0), stop=(kc == 7),
                    )
                poi = ps.tile([P, 512], F32, name="poi", tag="mb")
                for kc in range(8):
                    nc.tensor.matmul(
                        poi, vcat[:, kc, 128:256], estrip[:, qt * 8 + kc, :],
                        start=(kc == 0), stop=(kc == 7),
                    )
                oT = otp.tile([P, 2, 512], BF16, name="oT", tag="oT")
                nc.vector.tensor_copy(oT[:, 0, :], por)
                nc.vector.tensor_copy(oT[:, 1, :], poi)
                wo_pend.append((b, qt, oT, rec4))

            def attn_back(tail=False):
                if not wo_pend:
                    return
                b, qt, oT, rec4 = wo_pend.pop(0)
                ybuf = yp.tile([P, 4, 256], F32, name="ybuf", tag="ybuf")
                base = b * S + qt * 512
                yv = y[base : base + 512, :].rearrange("(a p) c -> p a c", p=P)
                for pq in range(2):
                    pyt = pyp.tile([P, 512], F32, name="pyt", tag="y")
                    for j in range(2):
                        qc = 2 * pq + j
                        qsub = slice(qc * 128, (qc + 1) * 128)
                        half = slice(j * 256, (j + 1) * 256)
                        nc.tensor.matmul(pyt[:, half], oT[:, 0, qsub], wt["oc1"], start=True, stop=False)
                        nc.tensor.matmul(pyt[:, half], oT[:, 1, qsub], wt["oc2"], start=False, stop=True)
                        if tail and j == 1:
                            # drain path: ACT is idle, take every other scale
                            nc.scalar.activation(
                                ybuf[:, qc, :], pyt[:, half], AF.Copy,
                                scale=rec4[:, 2 * qc : 2 * qc + 1],
                            )
                        else:
                            nc.vector.tensor_scalar_mul(
                                ybuf[:, qc, :], pyt[:, half], rec4[:, 2 * qc : 2 * qc + 1]
                            )
                    if tail:
                        # split the final writeback so the first half's DMA
                        # overlaps the second half's egress
                        nc.sync.dma_start(yv[:, 2 * pq : 2 * pq + 2, :],
                                          ybuf[:, 2 * pq : 2 * pq + 2, :])
                if not tail:
                    nc.sync.dma_start(yv, ybuf)

            ACT_OFF = 300   # scheduler-priority delay for sqrt/exp chunks

            def defer_act(strip, estrip):
                """Queue sqrt (in-place f32) + exp (f32 -> bf16 E-strip) as
                half-strip chunks, issued at artificially late scheduler
                priority so ready score-squares preempt them and they
                backfill ACT idle time instead."""
                def issue(fn, off):
                    with tc.high_priority(offset=-off):
                        fn()
                for i in range(2):
                    h = strip[:, 8 * i : 8 * i + 8, :]
                    act_pend.append(lambda h=h: issue(
                        lambda: nc.scalar.activation(h, h, AF.Sqrt, scale=1.0 / D), ACT_OFF))
                for i in range(2):
                    h = strip[:, 8 * i : 8 * i + 8, :]
                    e = estrip[:, 8 * i : 8 * i + 8, :]
                    act_pend.append(lambda h=h, e=e: issue(
                        lambda: nc.scalar.activation(e, h, AF.Exp), 150))

            pend = []
            for b in range(B):
                proj(b)
                # attention for batch b-2 launches here, BEFORE scores(b),
                # so the strip-buffer rotation never creates an ACT<->PE
                # dependency cycle through the shared ma/mb PSUM slots
                while pend and pend[0][0] <= b - 2:
                    attn_back()
                    attn_front(*pend.pop(0))
                strip = sp.tile([P, 16, 512], F32R, name="strip", tag="strip")
                estrip = ep.tile([P, 16, 512], BF16, name="estrip", tag="estrip")
                if b < B - 1:
                    xload(b + 1, 0)
                    scores(b, 0, strip)
                    xload(b + 1, 1)
                    pop_act(2)
                    scores(b, 1, strip)
                    pop_act(2)
                    defer_act(strip, estrip)
                    pend += [(b, 0, estrip), (b, 1, estrip)]
                else:
                    # last batch: per-qt halves so attn can start sooner
                    for qt in range(2):
                        scores(b, qt, strip)
                        if pend:
                            attn_back()
                            attn_front(*pend.pop(0))
                        while act_pend:
                            pop_act()
                        # queue this half's sqrt/exp; the very last half gets
                        # fine 2-slice chunks so the draining attention can
                        # chase them kc-by-kc
                        w = 2 if qt == 1 else 4
                        for i in range(8 // w):
                            h = strip[:, qt * 8 + w * i : qt * 8 + w * i + w, :]
                            act_pend.append(
                                lambda h=h: nc.scalar.activation(h, h, AF.Sqrt, scale=1.0 / D)
                            )
                        for i in range(8 // w):
                            h = strip[:, qt * 8 + w * i : qt * 8 + w * i + w, :]
                            e = estrip[:, qt * 8 + w * i : qt * 8 + w * i + w, :]
                            act_pend.append(lambda h=h, e=e: nc.scalar.activation(e, h, AF.Exp))
                        pend.append((b, qt, estrip))
            for item in pend:
                while act_pend and item[0] == B - 1:
                    pop_act()
                attn_back()
                attn_front(*item)
            while act_pend:
                pop_act()
            attn_back(tail=True)
            attn_back(tail=True)
    nc.finalize()
    return nc


_NC = None


def _get_nc():
    global _NC
    if _NC is None:
        _NC = build_nc()
    return _NC


def make_in_maps(inputs):
    """Shard full inputs into 8 per-core input maps (head h -> core h)."""
    f = np.float32
    xT = {}
    for src_nm, nm in (("q_r", "xqr"), ("q_i", "xqi"), ("k_r", "xkr"),
                       ("k_i", "xki"), ("v_r", "xvr"), ("v_i", "xvi")):
        xT[nm] = np.asarray(inputs[src_nm], f).reshape(BS, D).T
    # xall layout: [P, t(8), nm(6), 512]
    stack = np.stack([xT[nm].reshape(P, 8, 512) for nm in X_NAMES], axis=2)
    x16 = np.ascontiguousarray(
        stack.reshape(P, 8 * 6 * 512).astype(ml_dtypes.bfloat16)
    )

    Wq_r = np.asarray(inputs["Wq_r"], f)
    Wq_i = np.asarray(inputs["Wq_i"], f)
    Wk_r = np.asarray(inputs["Wk_r"], f)
    Wk_i = np.asarray(inputs["Wk_i"], f)
    Wv_r = np.asarray(inputs["Wv_r"], f)
    Wv_i = np.asarray(inputs["Wv_i"], f)
    Wo_r = np.asarray(inputs["Wo_r"], f)
    Wo_i = np.asarray(inputs["Wo_i"], f)

    in_maps = []
    for h in range(H):
        sl = slice(h * D, (h + 1) * D)
        w = {
            "wqr": Wq_r[sl].T, "wqi": Wq_i[sl].T, "nwqi": -Wq_i[sl].T,
            "wkr": Wk_r[sl].T, "wki": Wk_i[sl].T, "nwki": -Wk_i[sl].T,
            "vc1": np.concatenate([Wv_r[sl].T, Wv_i[sl].T], axis=1),
            "vc2": np.concatenate([-Wv_i[sl].T, Wv_r[sl].T], axis=1),
            "oc1": np.concatenate([Wo_r[:, sl].T, Wo_i[:, sl].T], axis=1),
            "oc2": np.concatenate([-Wo_i[:, sl].T, Wo_r[:, sl].T], axis=1),
        }
        wpack = np.zeros((P, WPACK_COLS), f)
        for nm, off in W1_OFF.items():
            wpack[:, off : off + P] = w[nm]
        for nm, off in W2_OFF.items():
            wpack[:, off : off + 2 * P] = w[nm]
        in_maps.append(
            {"xall": x16, "wpack": wpack.astype(ml_dtypes.bfloat16)}
        )
    return in_maps


def run(inputs, trace=False):
    nc = _get_nc()
    in_maps = make_in_maps(inputs)
    res = run_bass_kernel_spmd(nc, in_maps, core_ids=list(range(H)), trace=trace)
    ysum = np.zeros((BS, 2 * P), np.float64)
    for r in res.results:
        ysum += r["y"].astype(np.float64)
    yr = ysum[:, :P].reshape(B, S, D)
    yi = ysum[:, P:].reshape(B, S, D)
    out = (yr + 1j * yi).astype(np.complex64)
    return out, res


def kernel(**inputs):
    out, _ = run(inputs, trace=False)
    return out
